# revision 23
# baseline (speedup 1.0000x reference)
"""Trainium2 Bass kernel for nn_Disp_61125974557155.

Computes: trilinear upsample of a cost volume [B,1,48,64,128] ->
[B,193,256,512] (align_corners=False, edge-replicated), softmin over
disparity, disparity regression -> [B,256,512].

Design (per core; 8 cores = 2 batches x 4 H'-quarters). Output row
j = 4l+2+phi interpolates x-rows (l, l+1) with weight w_phi =
0.125+0.25*phi on row l+1 -- all four phi share one rhs per l, so the
(phi, d') axis flattens to 772 rows that pack into six FULL 128-row
chunks per l. Every ACT exp instruction covers 128 partitions x 1536
cols (one 3-bank PSUM tile), which is the ACT processing floor; ACT is
the bottleneck engine so everything else stays off its critical path.

  - DVE: W-axis 4x lerp at low res -> xsw [100, 17, 4, 128] bf16.
  - PE: D+H expansion folded in one bf16 matmul per (l, chunk):
    vol = amat[:, chunk]^T @ xsw_row(l+1) -> [128, 512] PSUM f32.
    Chunks are 128 consecutive rows of m = phi*193 + d', so chunk
    boundaries stay within at most two phi-segments.
  - The 4 leftover rows per l (phi3/phi1 d'189..192, taps k' 47..49
    only) are computed for ALL l by ONE K=102 matmul against a
    separately-loaded gather xg of those source rows -> prem [64, 512].
  - ACT: e = exp(-vol), one [128, 1536] instr per PSUM tile, bf16 out.
  - PE: flipped stats (e stationary, rmat moving, N=2..4) accumulate
    (S0, S1) = (sum e, sum d'*e) into one persistent pixel-major PSUM
    bank ps[s, (q, j, 2)]; block-diagonal rmat columns let one matmul
    cover both phi-segments of a chunk, and one K=64 matmul per q
    covers all 16 remainder pixel-rows.
  - DVE: out = S1 * recip(S0); PE transposes q-planes (reusing the
    prem bank); DMA out.
"""

import numpy as np
from contextlib import ExitStack

import concourse.bass as bass
import concourse.bacc as bacc
import concourse.tile as tile
from concourse import mybir
from concourse.bass_utils import run_bass_kernel_spmd
from concourse.tile_rust import add_dep_helper

F32 = mybir.dt.float32
BF16 = mybir.dt.bfloat16

MAXDISP = 192
DP = MAXDISP + 1      # 193 disparities
KD = 48               # low-res D
KP = KD + 2           # padded k' (edge-replicated)
NCORES = 8
NROW = 17             # xsw rows i = l+1 for l = -1..15
ROW_GROUPS = ((0, 1), (1, 1), (2, 1), (3, 1), (4, 2), (6, 2), (8, 3), (11, 3), (14, 3))
WW = (0.375, 0.125, 0.875, 0.625)   # W lerp: coef, (dc, hc) per rw
NREM = 66             # remainder rows: 2 + 15*4 (parts 0..61), l=15 at 64..65

# inner chunk column ranges of amat (m = phi*193 + d'), all M=128
N_CHUNK = 6
# segments per inner chunk: (phi, d'_start, nrows) -- for rmat + out js
CHUNK_SEGS = [
    [(0, 0, 128)],
    [(0, 128, 65), (1, 0, 63)],
    [(1, 63, 128)],
    [(1, 191, 2), (2, 0, 126)],
    [(2, 126, 67), (3, 0, 61)],
    [(3, 61, 128)],
]
# edge tiles reuse chunk types 0..2 (same d'-ranges, phi shifted)
EDGE_COL0 = {-1: 386, 15: 0}   # amat col offset of edge chunk 0
EDGE_PHI0 = {-1: 2, 15: 0}     # phi of the first segment


def _build_ad() -> np.ndarray:
    """A_D [193, 50]: D-axis linear upsample matrix on padded k' = k+1."""
    ad = np.zeros((DP, KP), dtype=np.float64)
    for dp in range(DP):
        i = (dp + 0.5) * KD / DP - 0.5
        fl = int(np.floor(i))
        fr = i - fl
        ad[dp, fl + 1] += 1.0 - fr
        ad[dp, fl + 2] += fr
    return ad


def _rem_blocks():
    """Remainder row blocks: (l, phi, [d' list]) in prem partition order."""
    blocks = [(-1, 3, [191, 192])]
    for l in range(15):
        blocks.append((l, 3, [189, 190, 191, 192]))
    blocks.append((15, 1, [191, 192]))
    return blocks


def _build_consts():
    bf = mybir.dt.np(BF16)
    ad = _build_ad()                      # [193, 50]
    # amat [100, 772]: col m = phi*193 + d'
    amat = np.zeros((2 * KP, 4 * DP), dtype=np.float64)
    for phi in range(4):
        w = 0.125 + 0.25 * phi
        amat[:KP, phi * DP : (phi + 1) * DP] = (1.0 - w) * ad.T
        amat[KP:, phi * DP : (phi + 1) * DP] = w * ad.T

    # rem amat [102, 66]: rows (l, half, k'-47) for k' in 47..49; cols per
    # _rem_blocks order -- l=15's block sits at cols 64..65 so the stat
    # matmul reading it starts at base partition 64 (hw constraint)
    rem_a = np.zeros((102, NREM), dtype=np.float64)
    col = 0
    for bi, (l, phi, ds) in enumerate(_rem_blocks()):
        if l == 15:
            col = 64
        w = 0.125 + 0.25 * phi
        r0 = 6 * (l + 1)
        for dp in ds:
            for k in range(3):
                rem_a[r0 + k, col] = (1.0 - w) * ad[dp, 47 + k]
                rem_a[r0 + 3 + k, col] = w * ad[dp, 47 + k]
            col += 1
    assert col == NREM

    # rmat [128, 52]: per-chunk-type column blocks of (1, d') pairs, rows
    # zero outside each segment; cols 36.. = remainder blocks on rows 0..63
    rmat = np.zeros((128, 52), dtype=np.float64)
    c = 0
    chunk_cols = []
    for segs in CHUNK_SEGS:
        cols = c
        p0 = 0
        for (_, d0, n) in segs:
            rmat[p0 : p0 + n, c] = 1.0
            rmat[p0 : p0 + n, c + 1] = d0 + np.arange(n)
            p0 += n
            c += 2
        chunk_cols.append((cols, c - cols))
    rem_cols = c
    p0 = 0
    for (l, phi, ds) in _rem_blocks():
        if l == 15:
            p0 = 64
        n = len(ds)
        rmat[p0 : p0 + n, c] = 1.0
        rmat[p0 : p0 + n, c + 1] = ds
        p0 += n
        c += 2
    assert c <= 52 and p0 == NREM

    ident = np.eye(128, dtype=np.float32)
    return (
        np.ascontiguousarray(amat, dtype=np.float32).astype(bf),
        np.ascontiguousarray(rem_a, dtype=np.float32).astype(bf),
        rmat.astype(np.float32).astype(bf),
        ident,
        chunk_cols,
        rem_cols,
    )


_CONSTS = _build_consts()


def _wlerp(nc, tmp_pool, out_ap, src_ap, kp, gn):
    """W-axis 4x lerp: out[p, g, rw, s] from src[p, g, 0:130] (padded).
    Shared difference d[s] = src[s] - src[s+1]; each rw plane is one
    fused op hi + c*d."""
    mult = mybir.AluOpType.mult
    add = mybir.AluOpType.add
    t_d = tmp_pool.tile([kp, gn, 129], F32, tag="wld")
    nc.vector.tensor_sub(t_d[0:kp], src_ap[:, :, 0:129], src_ap[:, :, 1:130])
    for rw, coef in enumerate(WW):
        dc = 0 if rw < 2 else 1
        hc = dc + 1
        nc.vector.scalar_tensor_tensor(
            out=out_ap[:, :, rw, :],
            in0=t_d[0:kp][:, :, dc : dc + 128],
            scalar=coef,
            in1=src_ap[:, :, hc : hc + 128],
            op0=mult,
            op1=add,
        )


def _build_nc() -> bass.Bass:
    amat_np, rem_a_np, rmat_np, ident_np, chunk_cols, rem_cols = _CONSTS

    nc = bacc.Bacc()
    xsd = nc.declare_dram_parameter("xsd", [2 * KP, NROW * 130], F32, isOutput=False)
    xg = nc.declare_dram_parameter("xg", [102, 130], F32, isOutput=False)
    amat = nc.declare_dram_parameter("amat", [2 * KP, 4 * DP], BF16, isOutput=False)
    rem_am = nc.declare_dram_parameter("rem_am", [102, NREM], BF16, isOutput=False)
    rmat = nc.declare_dram_parameter("rmat", [128, 52], BF16, isOutput=False)
    ident = nc.declare_dram_parameter("ident", [128, 128], F32, isOutput=False)
    outp = nc.declare_dram_parameter("out", [64, 512], F32, isOutput=True)

    xsd_v = xsd.rearrange("p (h w) -> p h w", h=NROW)
    exp_fn = mybir.ActivationFunctionType.Exp

    # Chain PE matmuls in emission order: guarantees the first stat matmul
    # (start=True) executes first and keeps the software pipeline order.
    last_pe = [None]

    def pe_matmul(*args, **kwargs):
        ins = nc.tensor.matmul(*args, **kwargs)
        if last_pe[0] is not None:
            add_dep_helper(ins.ins, last_pe[0].ins, False, "pe emission order")
        last_pe[0] = ins
        return ins

    def pe_transpose(*args, **kwargs):
        ins = nc.tensor.transpose(*args, **kwargs)
        if last_pe[0] is not None:
            add_dep_helper(ins.ins, last_pe[0].ins, False, "pe emission order")
        last_pe[0] = ins
        return ins

    with ExitStack() as ctx:
        tc = ctx.enter_context(tile.TileContext(nc))
        singles = ctx.enter_context(tc.tile_pool(name="singles", bufs=1))
        tmp_pool = ctx.enter_context(tc.tile_pool(name="tmp", bufs=4))
        epool = ctx.enter_context(tc.tile_pool(name="epool", bufs=6))
        fin = ctx.enter_context(tc.tile_pool(name="fin", bufs=1))
        pvol = ctx.enter_context(tc.tile_pool(name="pvol", bufs=3, space="PSUM"))
        pstat = ctx.enter_context(tc.tile_pool(name="pstat", bufs=1, space="PSUM"))
        prem_p = ctx.enter_context(tc.tile_pool(name="prem", bufs=1, space="PSUM"))

        # ---- input loads: xsd groups on the sync HWDGE queue (they gate
        # the lerp chain); constants + xg via gpsimd SWDGE in parallel ----
        s_xsd = []
        s_xg = singles.tile([102, 1, 130], F32, tag="xg")
        for g, (g0, gn) in enumerate(ROW_GROUPS):
            t_x = singles.tile([2 * KP, gn, 130], F32, tag=f"xsd{g}")
            nc.sync.dma_start(out=t_x, in_=xsd_v[:, g0 : g0 + gn, :])
            s_xsd.append(t_x)
            if g == 0:
                # xg early: it feeds the xgw lerp which gates the rem matmul
                nc.sync.dma_start(out=s_xg[:, 0, :], in_=xg[:, :])
        s_am = singles.tile([2 * KP, 4 * DP], BF16, tag="am")
        nc.gpsimd.dma_start(out=s_am, in_=amat[:, :])
        s_ram = singles.tile([102, NREM], BF16, tag="ram")
        nc.gpsimd.dma_start(out=s_ram, in_=rem_am[:, :])
        s_rm = singles.tile([128, 52], BF16, tag="rm")
        nc.gpsimd.dma_start(out=s_rm, in_=rmat[:, :])
        s_id = singles.tile([128, 128], F32, tag="id")
        nc.gpsimd.dma_start(out=s_id, in_=ident[:, :])

        # ---- W-axis 4x lerp -> bf16 (xgw right after group 0 so the rem
        # matmul never stalls the PE chain) ----
        s_xsw = []
        s_xgw = singles.tile([102, 1, 4, 128], BF16, tag="xgw")
        for g, (g0, gn) in enumerate(ROW_GROUPS):
            t_w = singles.tile([2 * KP, gn, 4, 128], BF16, tag=f"xsw{g}")
            _wlerp(nc, tmp_pool, t_w, s_xsd[g], 2 * KP, gn)
            s_xsw.append(t_w)
            if g == 0:
                _wlerp(nc, tmp_pool, s_xgw, s_xg, 102, 1)

        def xsw_row(i: int) -> bass.AP:
            for g, (g0, gn) in enumerate(ROW_GROUPS):
                if g0 <= i < g0 + gn:
                    return (
                        s_xsw[g][:, i - g0, :, :].rearrange("p q s -> p (q s)")
                    )
            raise IndexError(i)

        # preload the Exp activation table during the input DMA window so
        # the first real exp doesn't pay the 1.28us table load
        warm = fin.tile([1, 1], F32, tag="warm")
        nc.vector.memset(warm, 0.0)
        warm2 = fin.tile([1, 1], BF16, tag="warm2")
        nc.scalar.activation(warm2, warm, exp_fn, scale=-1.0)

        # ---- persistent pixel-major stats bank ----
        # ps[p=s, q, j, :] = (S0, S1) of output pixel (row j, w' = 4*s + q)
        ps = pstat.tile([128, 512], F32, tag="ps")
        ps_v = ps.rearrange("p (q j s) -> p q j s", q=4, s=2)
        # remainder vol rows -> one shared bank; later reused for transposes
        prem = prem_p.tile([128, 512], F32, tag="prem")

        stat_state = {"first": True}

        def stat_matmul(out_ap, lhsT, rhs, last=False):
            pe_matmul(
                out_ap,
                lhsT,
                rhs,
                start=stat_state["first"],
                stop=last,
                skip_group_check=True,
            )
            stat_state["first"] = False

        # stats of one step: list of (et, chunks) where chunks =
        # (col_in_tile, ctype, j0, nsegs)
        def emit_stats(batch):
            et, chunks = batch
            for (tc_off, ctype, j0, nsegs) in chunks:
                rc0, rcn = chunk_cols[ctype]
                for q in range(4):
                    stat_matmul(
                        ps_v[:, q, j0 : j0 + nsegs, :],
                        et[0:128, tc_off + 128 * q : tc_off + 128 * (q + 1)],
                        s_rm[0:128, rc0 : rc0 + rcn],
                    )

        # ---- main pipeline over l = -1, rem, 0..14, 15 ----
        # Each step: vol matmuls + ACT exp per tile; stats deferred one
        # step so PE never stalls on the current step's ACT.
        pending = []

        def emit_step_tiles(tiles):
            """tiles: list of (amat_col0, nchunks, chunk specs); 2-bank
            [128, 1024] PSUM tiles on a 3-deep ring so vol(l+1) overlaps
            ACT(l) instead of serializing on the buffer WAR."""
            nonlocal pending
            out = []
            for (a0, nch, specs) in tiles:
                pv = pvol.tile([128, 2 * 512], F32, tag="pv")
                et = epool.tile([128, 2 * 512], BF16, tag="e")
                i = specs["i"]
                rhs = xsw_row(i)
                for ci in range(nch):
                    pe_matmul(
                        pv[0:128, 512 * ci : 512 * (ci + 1)],
                        s_am[:, a0 + 128 * ci : a0 + 128 * (ci + 1)],
                        rhs,
                        start=True,
                        stop=True,
                    )
                nc.scalar.activation(
                    et[0:128, 0 : 512 * nch], pv[0:128, 0 : 512 * nch],
                    exp_fn, scale=-1.0,
                )
                out.append((et, specs["chunks"]))
            for b in pending:
                emit_stats(b)
            pending = out

        # edge l = -1 (phi2, phi3; j = 0, 1): chunk types 0..2
        emit_step_tiles([
            (386, 2, {
                "i": 0,
                "chunks": [(0, 0, 0, 1), (512, 1, 0, 2)],
            }),
            (642, 1, {
                "i": 0,
                "chunks": [(0, 2, 1, 1)],
            }),
        ])

        e_rem = fin.tile([NREM, 512], BF16, tag="erem")

        # ---- batched finalize: js 16B..16B+16 are final once stats of
        # step 4B+3 have run, so batches 0..2 overlap the main pipeline
        # and only batch 3 sits in the drain. Per batch: recip/mult on
        # DVE, PE transpose into the prem bank (free after e_rem), copy
        # to a per-batch om tile, DMA out. Transposes are NOT chained
        # into the PE emission order -- their data deps place them.
        rec = fin.tile([128, 4, 64], F32, tag="rec")
        oo = fin.tile([128, 4, 64], F32, tag="oo")
        prem_t = prem.rearrange("p (q s) -> p q s", q=4)

        def finalize_batch(bi):
            j0 = 16 * bi
            om = fin.tile([16, 128, 4], F32, tag=f"om{bi}")
            for q in range(4):
                nc.vector.reciprocal(
                    rec[:, q, j0 : j0 + 16], ps_v[:, q, j0 : j0 + 16, 0]
                )
                nc.vector.tensor_mul(
                    oo[:, q, j0 : j0 + 16],
                    ps_v[:, q, j0 : j0 + 16, 1],
                    rec[:, q, j0 : j0 + 16],
                )
                nc.tensor.transpose(
                    prem_t[0:16, q, 0:128], oo[:, q, j0 : j0 + 16], s_id
                )
                nc.vector.tensor_copy(om[:, :, q], prem_t[0:16, q, 0:128])
            eng = nc.sync if bi % 2 == 0 else nc.gpsimd
            eng.dma_start(
                out=outp[j0 : j0 + 16, :],
                in_=om.rearrange("j s q -> j (s q)"),
            )

        # inner l = 0..14: two tiles (chunks 0-2, 3-5)
        for l in range(15):
            jb = 4 * l + 2
            emit_step_tiles([
                (0, 2, {
                    "i": l + 1,
                    "chunks": [(0, 0, jb, 1), (512, 1, jb, 2)],
                }),
                (256, 2, {
                    "i": l + 1,
                    "chunks": [(0, 2, jb + 1, 1), (512, 3, jb + 1, 2)],
                }),
                (512, 2, {
                    "i": l + 1,
                    "chunks": [(0, 4, jb + 2, 2), (512, 5, jb + 3, 1)],
                }),
            ])
            if l == 0:
                # remainder rows for ALL l: one K=102 matmul -> prem
                pe_matmul(
                    prem[0:NREM, 0:512],
                    s_ram[:, :],
                    s_xgw[:, 0, :, :].rearrange("p q s -> p (q s)"),
                    start=True,
                    stop=True,
                )
                nc.scalar.activation(
                    e_rem, prem[0:NREM, 0:512], exp_fn, scale=-1.0
                )
            if l in (6, 10, 14):
                finalize_batch((l - 6) // 4)
            if l == 1:
                # rem stats early (e_rem ready): 16 strided js + j=63
                for q in range(4):
                    stat_matmul(
                        ps_v[:, q, 1:62:4, :],
                        e_rem[0:62, 128 * q : 128 * (q + 1)],
                        s_rm[0:62, rem_cols : rem_cols + 32],
                    )
                    stat_matmul(
                        ps_v[:, q, 63:64, :],
                        e_rem[64:66, 128 * q : 128 * (q + 1)],
                        s_rm[64:66, rem_cols + 32 : rem_cols + 34],
                    )

        # edge l = 15 (phi0, phi1; j = 62, 63)
        emit_step_tiles([
            (0, 2, {
                "i": 16,
                "chunks": [(0, 0, 62, 1), (512, 1, 62, 2)],
            }),
            (256, 1, {
                "i": 16,
                "chunks": [(0, 2, 63, 1)],
            }),
        ])
        # flush last stats (mark the very last one with stop=True)
        for ti, (et, chunks) in enumerate(pending):
            for k, (tc_off, ctype, j0, nsegs) in enumerate(chunks):
                rc0, rcn = chunk_cols[ctype]
                for q in range(4):
                    last = (
                        ti == len(pending) - 1
                        and k == len(chunks) - 1
                        and q == 3
                    )
                    stat_matmul(
                        ps_v[:, q, j0 : j0 + nsegs, :],
                        et[0:128, tc_off + 128 * q : tc_off + 128 * (q + 1)],
                        s_rm[0:128, rc0 : rc0 + rcn],
                        last=last,
                    )
        pending = []

        # ---- last finalize batch (js 48..63, after edge-15 stats) ----
        finalize_batch(3)

    nc.compile()
    return nc


_CACHE: dict = {}


def _shard_inputs(x: np.ndarray):
    """Edge-pad and slice per-core shards (memory movement only)."""
    bf = mybir.dt.np(BF16)
    xpad = np.pad(x[:, 0], ((0, 0), (1, 1), (1, 3), (1, 1)), mode="edge")
    amat_np, rem_a_np, rmat_np, ident_np, _, _ = _CONSTS
    in_maps = []
    for c in range(NCORES):
        b, q = divmod(c, 4)
        xs = xpad[b][:, 16 * q : 16 * q + 18, :]          # [50, 18, 130]
        xsd = np.concatenate([xs[:, 0:17, :], xs[:, 1:18, :]], axis=0)
        xsd = np.ascontiguousarray(
            xsd.reshape(2 * KP, NROW * 130), dtype=np.float32
        )
        # gather rows (l, half, k'-47) for the remainder matmul
        xg = np.empty((102, 130), dtype=np.float32)
        xg[0::6] = xsd.reshape(2 * KP, NROW, 130)[47, :, :]
        xg[1::6] = xsd.reshape(2 * KP, NROW, 130)[48, :, :]
        xg[2::6] = xsd.reshape(2 * KP, NROW, 130)[49, :, :]
        xg[3::6] = xsd.reshape(2 * KP, NROW, 130)[97, :, :]
        xg[4::6] = xsd.reshape(2 * KP, NROW, 130)[98, :, :]
        xg[5::6] = xsd.reshape(2 * KP, NROW, 130)[99, :, :]
        in_maps.append({
            "xsd": xsd,
            "xg": np.ascontiguousarray(xg),
            "amat": amat_np,
            "rem_am": rem_a_np,
            "rmat": rmat_np,
            "ident": ident_np,
        })
    return in_maps


def kernel(x: np.ndarray, _trace: bool = False, _tmpdir=None):
    x = np.asarray(x, dtype=np.float32)
    assert x.shape == (2, 1, 48, 64, 128), x.shape
    if "nc" not in _CACHE:
        _CACHE["nc"] = _build_nc()
    nc = _CACHE["nc"]
    in_maps = _shard_inputs(x)
    res = run_bass_kernel_spmd(
        nc, in_maps, list(range(NCORES)), trace=_trace, tmpdir=_tmpdir
    )
    out = np.zeros((2, 256, 512), dtype=np.float32)
    for c in range(NCORES):
        b, q = divmod(c, 4)
        out[b, 64 * q : 64 * (q + 1), :] = res.results[c]["out"]
    if _trace:
        return out, res
    return out
